# revision 33
# baseline (speedup 1.0000x reference)
"""MoE layer (top-2 of 8 experts, SwiGLU FFN) on 8 trn2 NeuronCores.

Strategy: expert parallelism, one expert per core. The host computes only the
top-2 *selection* (index lists) and performs dispatch/combine data movement
(gather tokens per expert / scatter-add partial outputs); all floating-point
math that produces output values — gate logits, top-2 softmax weights, the
SwiGLU FFN — runs on device.

v4 over the original baseline:
  - bf16 operands everywhere on the PE (same 1 cycle/row as fp32r, half the
    DMA/SBUF traffic, and measured per-instruction overhead drops to ~0:
    512-col matmuls run 216ns vs 227ns fp32r, 128-col run 56ns vs 113ns);
    psum accumulation stays fp32.
  - single token chunk: x and one F-half of h stay resident in SBUF, so each
    weight matrix streams from HBM exactly once.
  - host pre-shuffles x/w1/w3/w2/gw into the SBUF-partition-major layout so
    every DMA descriptor is a contiguous run.
  - no `valid` mask: padded token columns are all-zero => h = 0 => y = 0
    regardless of the (garbage) gate weight computed for them.
  - gate weight broadcast via one TT-wide selector matmul per tile instead of
    four 128-wide ones.
  - gating chains and selector matmuls are interleaved into early phase-A
    f-blocks so the in-order PE neither stalls on the x-tile DMAs at startup
    (gating tile t is emitted only once its x tile has had time to land) nor
    head-of-line blocks on the gating DVE transposes.

Capping the capacity at 2048 by dropping overflow pairs was measured and
rejected: top-2 softmax gate weights on this input are never negligible
(min 0.034 across all 16384 pairs), so dropping the 135 overflow pairs costs
2.7e-2 relative error — over the 2e-2 gate.
"""

import numpy as np

T, D, F, E = 8192, 1024, 4096, 8
NCORES = 8
P = 128
TOK_TILE = 512

_nc_cache: dict = {}


def _build(C: int):
    """Build + compile the per-core Bass program for capacity C (multiple of 128)."""
    from contextlib import ExitStack

    import concourse.tile as tile
    from concourse import bacc, mybir
    from concourse.bass import ds

    f32 = mybir.dt.float32
    bf16 = mybir.dt.bfloat16
    KD, KF = D // P, F // P
    KH = KF // 2
    X = mybir.AxisListType.X
    Silu = mybir.ActivationFunctionType.Silu
    Tanh = mybir.ActivationFunctionType.Tanh
    Exp = mybir.ActivationFunctionType.Exp
    Alu = mybir.AluOpType

    nc = bacc.Bacc(
        "TRN2", target_bir_lowering=False, debug=False, num_devices=NCORES
    )
    xt = nc.dram_tensor("xt", [P, KD, C], bf16, kind="ExternalInput")
    gw = nc.dram_tensor("gw", [P, KD, E], bf16, kind="ExternalInput")
    w1 = nc.dram_tensor("w1", [KF, P, KD, P], bf16, kind="ExternalInput")
    w3 = nc.dram_tensor("w3", [KF, P, KD, P], bf16, kind="ExternalInput")
    w2 = nc.dram_tensor("w2", [KD, P, KF, P], bf16, kind="ExternalInput")
    yt = nc.dram_tensor("yt", [KD, P, C], bf16, kind="ExternalOutput")
    # second F-half partials go to their own tensor; the host adds them.
    # (DMA-accumulate into yt would read-modify-write DRAM on the kernel's
    # critical tail.)
    yt2 = nc.dram_tensor("yt2", [KD, P, C], bf16, kind="ExternalOutput")

    # token tiles: 512s, remainder (multiple of 128) last
    tiles = []
    t0 = 0
    while t0 + TOK_TILE <= C:
        tiles.append((t0, TOK_TILE))
        t0 += TOK_TILE
    if t0 < C:
        tiles.append((t0, C - t0))

    with ExitStack() as ctx:
        tc = ctx.enter_context(tile.TileContext(nc))
        const = ctx.enter_context(tc.tile_pool(name="const", bufs=1))
        xp = ctx.enter_context(tc.tile_pool(name="xp", bufs=1))
        wp = ctx.enter_context(tc.tile_pool(name="wp", bufs=4))
        hp = ctx.enter_context(tc.tile_pool(name="hp", bufs=1))
        yp = ctx.enter_context(tc.tile_pool(name="yp", bufs=4))
        gp = ctx.enter_context(tc.tile_pool(name="gp", bufs=2))
        psA = ctx.enter_context(tc.tile_pool(name="psA", bufs=2, space="PSUM"))
        psG = ctx.enter_context(tc.tile_pool(name="psG", bufs=1, space="PSUM"))
        psB = ctx.enter_context(tc.tile_pool(name="psB", bufs=3, space="PSUM"))

        # constants
        gw_sb = const.tile([P, KD, E], bf16)
        nc.sync.dma_start(gw_sb[:], gw[:, :, :])
        # selector rows: picks partition 0 of the rhs in the broadcast matmul
        sel_sb = const.tile([32, P], bf16)
        nc.vector.memset(sel_sb[:], 0.0)
        nc.vector.memset(sel_sb[0:1, :], 1.0)

        # x in two half-loads: C/2-long runs per (partition, kd) keep DMA
        # descriptors >=2KB (per-512-tile loads ran at half DMA efficiency;
        # finer 640-column chunks also measured slower than halves)
        x_sb = xp.tile([P, KD, C], bf16, tag="x", name="x")
        CH = (C // 2) // P * P
        nc.sync.dma_start(x_sb[:, :, ds(0, CH)], xt[:, :, ds(0, CH)])
        nc.sync.dma_start(x_sb[:, :, ds(CH, C - CH)], xt[:, :, ds(CH, C - CH)])
        wb_all = xp.tile([P, C], f32, tag="wb_all", name="wba")

        # PE warm-up during the initial x DMA: ramps the p-state and keeps
        # the in-order PE off the x-dependent gating until x has landed
        for wi in range(16):
            warm = psG.tile([E, E], f32, tag="g", name=f"warm_{wi}")
            nc.tensor.matmul(
                warm[:], gw_sb[:, wi % KD, :], gw_sb[:, wi % KD, :],
                start=True, stop=True,
            )

        wrt_tiles = []

        def emit_gating(t0, TT):
            """Top-2 softmax weight of own expert for one token tile; leaves
            the transposed weight row in wrt_tiles for the selector matmul."""
            S = TT // P
            lt_ps = psG.tile([E, TT], f32, tag="g", name=f"lt_{t0}")
            for kd in range(KD):
                nc.tensor.matmul(
                    lt_ps[:],
                    gw_sb[:, kd, :],
                    x_sb[:, kd, ds(t0, TT)],
                    start=(kd == 0),
                    stop=(kd == KD - 1),
                )
            lt32 = gp.tile([32, TT], f32, tag="lt32", name=f"lt32_{t0}")
            nc.vector.memset(lt32[:], 0.0)
            nc.vector.tensor_copy(lt32[0:E, :], lt_ps[:])
            lg = gp.tile([P, S, 32], f32, tag="lg", name=f"lg_{t0}")
            for s in range(S):
                for j in range(4):
                    nc.vector.transpose(
                        lg[ds(32 * j, 32), s],
                        lt32[:, ds(s * P + 32 * j, 32)],
                    )
            L = lg[:, :, 0:E]
            m1 = gp.tile([P, S, 1], f32, tag="m1", name=f"m1_{t0}")
            nc.vector.reduce_max(m1[:], L, axis=X)
            dd = gp.tile([P, S, E], f32, tag="d", name=f"d_{t0}")
            nc.vector.tensor_tensor(
                dd[:], L, m1[:].to_broadcast((P, S, E)), Alu.subtract
            )
            msk = gp.tile([P, S, E], f32, tag="msk", name=f"msk_{t0}")
            nc.vector.tensor_scalar(msk[:], dd[:], 0.0, None, Alu.is_ge)
            nc.vector.tensor_scalar(msk[:], msk[:], -100000.0, None, Alu.mult)
            nc.vector.tensor_add(msk[:], msk[:], dd[:])
            m2 = gp.tile([P, S, 1], f32, tag="m2", name=f"m2_{t0}")
            nc.vector.reduce_max(m2[:], msk[:], axis=X)
            # device m2 is RELATIVE (m2-m1, the mask adds dd): z = 2*dd0 - m2rel
            # = 2*l0 - m1 - m2true;  w = sigmoid(z) = 0.5 + 0.5*tanh(z/2)
            z = gp.tile([P, S, 1], f32, tag="z", name=f"z_{t0}")
            nc.vector.tensor_scalar(z[:], dd[:, :, 0:1], 2.0, None, Alu.mult)
            nc.vector.tensor_tensor(z[:], z[:], m2[:], Alu.subtract)
            th = gp.tile([P, S, 1], f32, tag="th", name=f"th_{t0}")
            nc.scalar.activation(th[:], z[:], Tanh, scale=0.5)
            wgt = gp.tile([P, S, 1], f32, tag=f"wgt{t0}", name=f"wgt_{t0}")
            nc.vector.tensor_scalar(wgt[:], th[:], 1.0, 0.5, Alu.add, Alu.mult)

            # wrt[32, TT]: row 0 carries the per-token weight, transposed
            wrt = gp.tile([32, TT], bf16, tag=f"wrt{t0}", name=f"wrt_{t0}")
            wcol = gp.tile([P, 32], bf16, tag="wcol", name=f"wcol_{t0}")
            for s in range(S):
                nc.vector.memset(wcol[:, 1:32], 0.0)
                nc.vector.tensor_copy(wcol[:, 0:1], wgt[:, s])
                for j in range(4):
                    nc.vector.transpose(
                        wrt[:, ds(s * P + 32 * j, 32)],
                        wcol[ds(32 * j, 32), :],
                    )
            wrt_tiles.append((t0, TT, wrt))

        def emit_sel():
            t0, TT, wrt = wrt_tiles.pop(0)
            wb_ps = psG.tile([P, TT], f32, tag="g", name=f"wbps_{t0}")
            nc.tensor.matmul(wb_ps[:], sel_sb[:], wrt[:], start=True, stop=True)
            nc.vector.tensor_copy(wb_all[:, ds(t0, TT)], wb_ps[:])

        # gating for the first two tiles leads; the rest interleave into
        # phase A so the PE is never waiting on an x-tile DMA
        gpend = list(tiles)
        emit_gating(*gpend.pop(0))
        if gpend:
            emit_gating(*gpend.pop(0))

        for fh in range(2):
            # ---- phase A: h(F-half) = silu(w1.T x) * (w3.T x) ----
            h_sb = hp.tile([P, KH, C], bf16, tag="h", name=f"h_{fh}")
            for fl in range(KH):
                f = fh * KH + fl
                w1_sb = wp.tile([P, KD, P], bf16, tag="w1", name=f"w1_{f}")
                nc.sync.dma_start(w1_sb[:], w1[f])
                w3_sb = wp.tile([P, KD, P], bf16, tag="w3", name=f"w3_{f}")
                nc.sync.dma_start(w3_sb[:], w3[f])
                for t0, TT in tiles:
                    h1 = psA.tile([P, TT], f32, tag="h1", name=f"ph1_{t0}_{f}")
                    h3 = psA.tile([P, TT], f32, tag="h3", name=f"ph3_{t0}_{f}")
                    for kd in range(KD):
                        nc.tensor.matmul(
                            h1[:],
                            w1_sb[:, kd, :],
                            x_sb[:, kd, ds(t0, TT)],
                            start=(kd == 0),
                            stop=(kd == KD - 1),
                        )
                    for kd in range(KD):
                        nc.tensor.matmul(
                            h3[:],
                            w3_sb[:, kd, :],
                            x_sb[:, kd, ds(t0, TT)],
                            start=(kd == 0),
                            stop=(kd == KD - 1),
                        )
                    s1 = gp.tile([P, TT], f32, tag="s1", name=f"s1_{t0}_{f}")
                    nc.scalar.activation(s1[:], h1[:], Silu)
                    nc.vector.tensor_mul(h_sb[:, fl, ds(t0, TT)], s1[:], h3[:])
                # remaining gating chains, then selector matmuls, one per slot
                if fh == 0:
                    if gpend:
                        emit_gating(*gpend.pop(0))
                    elif wrt_tiles:
                        emit_sel()

            # ---- phase B: yT(+=) (w2-half.T @ h) * wb ----
            for dm in range(KD):
                w2_sb = wp.tile([P, KH, P], bf16, tag="w2", name=f"w2_{fh}_{dm}")
                nc.sync.dma_start(w2_sb[:], w2[dm, :, ds(fh * KH, KH), :])
                for ti, (t0, TT) in enumerate(tiles):
                    yps = psB.tile([P, TT], f32, tag="y", name=f"y_{t0}_{fh}_{dm}")
                    for fk in range(KH):
                        nc.tensor.matmul(
                            yps[:],
                            w2_sb[:, fk, :],
                            h_sb[:, fk, ds(t0, TT)],
                            start=(fk == 0),
                            stop=(fk == KH - 1),
                        )
                    y_sb = yp.tile(
                        [P, TT], bf16, tag="y_sb", name=f"ysb_{t0}_{fh}_{dm}"
                    )
                    nc.vector.tensor_mul(y_sb[:], yps[:], wb_all[:, ds(t0, TT)])
                    dst = yt if fh == 0 else yt2
                    # final dm-block's writes ride the (idle) sync ring so the
                    # gpsimd ring drains before the exit barrier
                    last = fh == 1 and dm == KD - 1
                    eng = nc.sync if (last and ti % 2 == 0) else nc.gpsimd
                    eng.dma_start(dst[dm, :, ds(t0, TT)], y_sb[:])

    nc.compile()
    return nc


def _route(x: np.ndarray, gw: np.ndarray):
    """Top-2 expert selection (host; indices only — no output values)."""
    logits = x @ gw
    n = x.shape[0]
    top1 = np.argmax(logits, axis=1)
    l2 = logits.copy()
    l2[np.arange(n), top1] = -np.inf
    top2 = np.argmax(l2, axis=1)
    idx = [
        np.nonzero((top1 == e) | (top2 == e))[0].astype(np.int64)
        for e in range(gw.shape[1])
    ]
    return idx


def _shuffle_w13(w: np.ndarray):
    # [D, F] -> [KF, P, KD, P] partition-major blocks
    KD, KF = D // P, F // P
    return np.ascontiguousarray(w.reshape(KD, P, KF, P).transpose(2, 1, 0, 3))


def _shuffle_w2(w: np.ndarray):
    # [F, D] -> [KD, P, KF, P]
    KD, KF = D // P, F // P
    return np.ascontiguousarray(w.reshape(KF, P, KD, P).transpose(2, 1, 0, 3))


def kernel(x, gate_w, w1, w2, w3, _trace=False, _trace_cores=None, _result_box=None):
    import ml_dtypes
    from concourse.bass_utils import run_bass_kernel_spmd

    bf16 = ml_dtypes.bfloat16
    KD = D // P

    x = np.ascontiguousarray(np.asarray(x, dtype=np.float32))
    gw = np.ascontiguousarray(np.asarray(gate_w, dtype=np.float32))
    w1 = np.ascontiguousarray(np.asarray(w1, dtype=np.float32))
    w2 = np.ascontiguousarray(np.asarray(w2, dtype=np.float32))
    w3 = np.ascontiguousarray(np.asarray(w3, dtype=np.float32))
    assert x.shape == (T, D) and gw.shape == (D, E), (x.shape, gw.shape)
    assert w1.shape == (E, D, F) and w3.shape == (E, D, F), (w1.shape,)
    assert w2.shape == (E, F, D), (w2.shape,)

    idx = _route(x, gw)
    maxn = max(len(i) for i in idx)
    C = max(P, -(-maxn // P) * P)

    if C not in _nc_cache:
        _nc_cache[C] = _build(C)
    nc = _nc_cache[C]

    rot = np.arange(E)
    in_maps = []
    for e in range(E):
        n = len(idx[e])
        # x gather -> [P, KD, C] partition-major
        xg = x[idx[e]].astype(bf16)                       # [n, D]
        xt = np.zeros((P, KD, C), bf16)
        xt[:, :, :n] = xg.reshape(n, KD, P).transpose(2, 1, 0)
        gwr = np.ascontiguousarray(gw[:, (rot + e) % E]).astype(bf16)
        in_maps.append(
            {
                "xt": xt,
                "gw": np.ascontiguousarray(
                    gwr.reshape(KD, P, E).transpose(1, 0, 2)
                ),
                "w1": _shuffle_w13(w1[e].astype(bf16)),
                "w3": _shuffle_w13(w3[e].astype(bf16)),
                "w2": _shuffle_w2(w2[e].astype(bf16)),
            }
        )

    res = run_bass_kernel_spmd(
        nc,
        in_maps,
        core_ids=list(range(NCORES)),
        trace=_trace,
        trace_cores=_trace_cores,
    )
    if _result_box is not None:
        _result_box.append(res)

    out = np.zeros((T, D), np.float32)
    for e in range(E):
        n = len(idx[e])
        yt = np.asarray(res.results[e]["yt"])             # [KD, P, C] bf16
        yt2 = np.asarray(res.results[e]["yt2"])
        out[idx[e]] += (
            yt[:, :, :n].astype(np.float32) + yt2[:, :, :n].astype(np.float32)
        ).reshape(D, n).T
    return out
